# revision 2
# baseline (speedup 1.0000x reference)
"""Trainium2 Bass kernel for the Dipole GNN message-passing layer.

Strategy (8 NeuronCores):
  * Node MLPs (q, q2) are computed sharded over nodes (6272 nodes/core,
    host supplies x pre-transposed per shard), then the per-node feature
    table qq = [q | q2] (fp32, [50176, 64]) is AllGathered so every core
    holds the full table in DRAM.
  * Edges are sharded by dst range: host sorts edges by dst, so each core
    owns all edges landing in its 6272-node range and produces a disjoint
    slice of the output (no all-reduce needed).
  * Within a core, edges are grouped into 49 blocks of 128 dst nodes and
    padded per block to a fixed tile count. Each 128-edge tile:
      - gathers qq[src] rows via indirect DMA (128 rows/call),
      - builds M12[e, k*64+b*32+d] = vij[e,k]*c(rij[e])*qq[src[e], b*32+d]
        with three per-partition tensor_scalar ops,
      - builds a one-hot dst-selection matrix via is_equal against an
        iota row, and accumulates onehot.T @ M12 into the block's PSUM
        tile ([128 nodes, 192]).
  * Cross product + mix Dense(3,1) run as bulk DVE ops; host reassembles
    the [N, 32, 3] output from the per-core slices.
"""

import sys

sys.path.insert(0, "/opt/trn_rl_repo")

import numpy as np

import concourse.bass as bass
import concourse.mybir as mybir
import concourse.tile as tile
from concourse.bass_utils import run_bass_kernel_spmd

N, E, F, D = 50000, 800000, 128, 32
CUTOFF = 5.0
NCORES = 8
NPD = 6272          # nodes per device (49 blocks of 128)
NB = NPD // 128     # 49 blocks
NFULL = NPD * NCORES  # 50176
DQ = 2 * D          # 64 = [q | q2]

LAST_RUN_INFO = {}


def _split_sync_waits(nc, limit=1):
    """The walrus build here encodes very few sync waits per instruction
    (Drain: 1). Hoist excess on_wait entries onto single-wait NoOps
    inserted just before the instruction on the same engine."""
    ctr = 0
    for fn in nc.m.functions:
        for blk in fn.blocks:
            out = []
            for ins in blk.instructions:
                si = ins.sync_info
                if si is not None and len(si.on_wait) > limit:
                    waits = list(si.on_wait)
                    for w in waits[limit:]:
                        ctr += 1
                        nop = mybir.InstNoOp(
                            name=f"I-swfix-{ctr}", ins=[], outs=[]
                        )
                        nop.engine = ins.engine
                        nop.sync_info = mybir.SyncInfo(
                            on_wait=[w], on_update=[]
                        )
                        out.append(nop)
                    si.on_wait = waits[:limit]
                out.append(ins)
            blk.instructions = out
    return nc


def _build_nc(t_blk):
    NT = NB * t_blk  # tiles per device
    f32 = mybir.dt.float32
    nc = bass.Bass()

    xT = nc.dram_tensor("xT", [F, NPD], f32, kind="ExternalInput")
    w1 = nc.dram_tensor("w1", [F, F], f32, kind="ExternalInput")
    w1b = nc.dram_tensor("w1b", [F, F], f32, kind="ExternalInput")
    w2 = nc.dram_tensor("w2", [F, D], f32, kind="ExternalInput")
    w2b = nc.dram_tensor("w2b", [F, D], f32, kind="ExternalInput")
    b1c = nc.dram_tensor("b1c", [F, 1], f32, kind="ExternalInput")
    b1bc = nc.dram_tensor("b1bc", [F, 1], f32, kind="ExternalInput")
    b2bc = nc.dram_tensor("b2bc", [128, DQ], f32, kind="ExternalInput")
    wbmix = nc.dram_tensor("wbmix", [128, 6], f32, kind="ExternalInput")
    iota = nc.dram_tensor("iota", [128, 128], f32, kind="ExternalInput")
    srcT = nc.dram_tensor("srcT", [128, NT], mybir.dt.int32, kind="ExternalInput")
    dstmT = nc.dram_tensor("dstmT", [128, NT], f32, kind="ExternalInput")
    rijT = nc.dram_tensor("rijT", [128, NT], f32, kind="ExternalInput")
    vijT = nc.dram_tensor("vijT", [128, 3 * NT], f32, kind="ExternalInput")
    outd = nc.dram_tensor("outd", [128, NB * 96], f32, kind="ExternalOutput")

    with tile.TileContext(nc) as tc:
        with tc.tile_pool(name="dram", bufs=1, space="DRAM") as dram, \
             tc.tile_pool(name="persist", bufs=1) as pp:
            qq_own = dram.tile([NPD, DQ], f32)
            qq_full = dram.tile([NFULL, DQ], f32, addr_space="Shared")

            # ---- persistent SBUF loads (edge arrays + constants) ----
            src_sb = pp.tile([128, NT], mybir.dt.int32)
            dstm_sb = pp.tile([128, NT], f32)
            rij_sb = pp.tile([128, NT], f32)
            vij_sb = pp.tile([128, 3 * NT], f32)
            iota_sb = pp.tile([128, 128], f32)
            b2_sb = pp.tile([128, DQ], f32)
            wb_sb = pp.tile([128, 6], f32)
            nc.sync.dma_start(out=src_sb[:], in_=srcT[:])
            nc.sync.dma_start(out=dstm_sb[:], in_=dstmT[:])
            nc.sync.dma_start(out=rij_sb[:], in_=rijT[:])
            nc.sync.dma_start(out=vij_sb[:], in_=vijT[:])
            nc.sync.dma_start(out=iota_sb[:], in_=iota[:])
            nc.sync.dma_start(out=b2_sb[:], in_=b2bc[:])
            nc.sync.dma_start(out=wb_sb[:], in_=wbmix[:])

            # vijc[p, k*NT+g] = vij * c(rij), shared by both branches
            vijc_sb = pp.tile([128, 3 * NT], f32)
            c_sb = pp.tile([128, NT], f32)
            mask_sb = pp.tile([128, NT], f32)
            mu_sb = pp.tile([128, NB * 192], f32)
            out_sb = pp.tile([128, NB * 96], f32)

            # ---- stage 1: q/q2 for own node slice ----
            with tc.tile_pool(name="s1", bufs=1) as s1, \
                 tc.tile_pool(name="s1p", bufs=2, space="PSUM") as s1p, \
                 tc.tile_pool(name="s1w", bufs=3) as s1w:
                xT_sb = s1.tile([F, NPD], f32)
                h1_sb = s1.tile([F, NPD], f32)
                h2_sb = s1.tile([F, NPD], f32)
                w1_sb = s1.tile([F, F], f32, tag="w1")
                w1b_sb = s1.tile([F, F], f32, tag="w1b")
                w2_sb = s1.tile([F, D], f32, tag="w2")
                w2b_sb = s1.tile([F, D], f32, tag="w2b")
                b1_sb = s1.tile([F, 1], f32, tag="b1")
                b1b_sb = s1.tile([F, 1], f32, tag="b1b")
                nc.sync.dma_start(out=xT_sb[:], in_=xT[:])
                nc.sync.dma_start(out=w1_sb[:], in_=w1[:])
                nc.sync.dma_start(out=w1b_sb[:], in_=w1b[:])
                nc.sync.dma_start(out=w2_sb[:], in_=w2[:])
                nc.sync.dma_start(out=w2b_sb[:], in_=w2b[:])
                nc.sync.dma_start(out=b1_sb[:], in_=b1c[:])
                nc.sync.dma_start(out=b1b_sb[:], in_=b1bc[:])

                silu = mybir.ActivationFunctionType.Silu
                # layer 1, both branches: hT = silu(W.T @ xT + b)
                for wsb, bsb, hsb in ((w1_sb, b1_sb, h1_sb),
                                      (w1b_sb, b1b_sb, h2_sb)):
                    c0 = 0
                    while c0 < NPD:
                        cw = min(512, NPD - c0)
                        ph = s1p.tile([128, 512], f32, tag="ph")
                        nc.tensor.matmul(
                            out=ph[:, :cw], lhsT=wsb[:],
                            rhs=xT_sb[:, c0:c0 + cw],
                            start=True, stop=True,
                        )
                        nc.scalar.activation(
                            out=hsb[:, c0:c0 + cw], in_=ph[:, :cw],
                            func=silu, bias=bsb[:],
                        )
                        c0 += cw
                # layer 2 per 128-node chunk -> qq rows
                for cnode in range(NB):
                    sl = slice(cnode * 128, (cnode + 1) * 128)
                    pq = s1p.tile([128, DQ], f32, tag="pq")
                    nc.tensor.matmul(out=pq[:, 0:D], lhsT=h1_sb[:, sl],
                                     rhs=w2_sb[:], start=True, stop=True)
                    nc.tensor.matmul(out=pq[:, D:DQ], lhsT=h2_sb[:, sl],
                                     rhs=w2b_sb[:], start=True, stop=True)
                    zq = s1w.tile([128, DQ], f32, tag="zq")
                    qt = s1w.tile([128, DQ], f32, tag="qt")
                    nc.vector.tensor_tensor(out=zq[:], in0=pq[:],
                                            in1=b2_sb[:],
                                            op=mybir.AluOpType.add)
                    nc.scalar.activation(out=qt[:], in_=zq[:], func=silu)
                    nc.sync.dma_start(out=qq_own[sl, :], in_=qt[:])

            # ---- distribute qq to all cores ----
            nc.gpsimd.collective_compute(
                "AllGather", mybir.AluOpType.bypass,
                replica_groups=[list(range(NCORES))],
                ins=[qq_own.opt()], outs=[qq_full.opt()],
            )

            # ---- edge-stage prep: cutoff + vij*c ----
            half = NT // 2
            for s0, s1e in ((0, half), (half, NT)):
                nc.scalar.activation(
                    out=c_sb[:, s0:s1e], in_=rij_sb[:, s0:s1e],
                    func=mybir.ActivationFunctionType.Sin,
                    scale=wb_sb[:, 5:6], bias=wb_sb[:, 4:5],
                )
            nc.vector.tensor_scalar(
                out=mask_sb[:], in0=rij_sb[:], scalar1=CUTOFF, scalar2=None,
                op0=mybir.AluOpType.is_lt,
            )
            nc.vector.tensor_scalar(
                out=c_sb[:], in0=c_sb[:], scalar1=1.0, scalar2=0.5,
                op0=mybir.AluOpType.add, op1=mybir.AluOpType.mult,
            )
            nc.vector.tensor_tensor(out=c_sb[:], in0=c_sb[:], in1=mask_sb[:],
                                    op=mybir.AluOpType.mult)
            for k in range(3):
                nc.vector.tensor_tensor(
                    out=vijc_sb[:, k * NT:(k + 1) * NT],
                    in0=vij_sb[:, k * NT:(k + 1) * NT],
                    in1=c_sb[:], op=mybir.AluOpType.mult,
                )

            # ---- edge loop: gather + one-hot matmul segment sum ----
            with tc.tile_pool(name="ep", bufs=6) as ep, \
                 tc.tile_pool(name="epp", bufs=2, space="PSUM") as epp:
                for b in range(NB):
                    pmu = epp.tile([128, 192], f32, tag="pmu")
                    for t in range(t_blk):
                        g = b * t_blk + t
                        qg = ep.tile([128, DQ], f32, tag="qg")
                        nc.gpsimd.indirect_dma_start(
                            out=qg[:], out_offset=None, in_=qq_full[:],
                            in_offset=bass.IndirectOffsetOnAxis(
                                ap=src_sb[:, g:g + 1], axis=0),
                        )
                        oh = ep.tile([128, 128], f32, tag="oh")
                        nc.vector.tensor_scalar(
                            out=oh[:], in0=iota_sb[:],
                            scalar1=dstm_sb[:, g:g + 1], scalar2=None,
                            op0=mybir.AluOpType.is_equal,
                        )
                        m12 = ep.tile([128, 192], f32, tag="m12")
                        for k in range(3):
                            nc.vector.tensor_scalar(
                                out=m12[:, k * DQ:(k + 1) * DQ], in0=qg[:],
                                scalar1=vijc_sb[:, k * NT + g:k * NT + g + 1],
                                scalar2=None, op0=mybir.AluOpType.mult,
                            )
                        nc.tensor.matmul(
                            out=pmu[:], lhsT=oh[:], rhs=m12[:],
                            start=(t == 0), stop=(t == t_blk - 1),
                        )
                    nc.scalar.copy(out=mu_sb[:, b * 192:(b + 1) * 192],
                                   in_=pmu[:])

            # ---- finalize: cross product + mix ----
            with tc.tile_pool(name="fin", bufs=1) as fin:
                mu3v = mu_sb[:].rearrange("p (g c) -> p g c", c=192)
                out3v = out_sb[:].rearrange("p (g c) -> p g c", c=96)

                def muv(k):
                    return mu3v[:, :, k * DQ:k * DQ + D]

                def mu2v(k):
                    return mu3v[:, :, k * DQ + D:k * DQ + DQ]

                W = NB * D
                for k in range(3):
                    k1, k2 = (k + 1) % 3, (k + 2) % 3
                    t1 = fin.tile([128, W], f32, tag="t1")
                    t2 = fin.tile([128, W], f32, tag="t2")
                    o3 = fin.tile([128, W], f32, tag="o3")
                    o1 = fin.tile([128, W], f32, tag="o1")
                    o2 = fin.tile([128, W], f32, tag="o2")
                    mul = mybir.AluOpType.mult
                    nc.vector.tensor_tensor(out=t1[:], in0=muv(k1), in1=mu2v(k2), op=mul)
                    nc.vector.tensor_tensor(out=t2[:], in0=muv(k2), in1=mu2v(k1), op=mul)
                    nc.vector.tensor_tensor(out=t1[:], in0=t1[:], in1=t2[:],
                                            op=mybir.AluOpType.subtract)
                    # o3 = w2*mu3 + b ; o1 = w0*mu ; o2 = w1*mu2
                    nc.vector.tensor_scalar(out=o3[:], in0=t1[:],
                                            scalar1=wb_sb[:, 2:3],
                                            scalar2=wb_sb[:, 3:4],
                                            op0=mul, op1=mybir.AluOpType.add)
                    nc.vector.tensor_scalar(out=o1[:], in0=muv(k),
                                            scalar1=wb_sb[:, 0:1],
                                            scalar2=None, op0=mul)
                    nc.vector.tensor_scalar(out=o2[:], in0=mu2v(k),
                                            scalar1=wb_sb[:, 1:2],
                                            scalar2=None, op0=mul)
                    nc.vector.tensor_tensor(out=o1[:], in0=o1[:], in1=o2[:],
                                            op=mybir.AluOpType.add)
                    nc.vector.tensor_tensor(out=out3v[:, :, k * D:(k + 1) * D],
                                            in0=o1[:], in1=o3[:],
                                            op=mybir.AluOpType.add)
                nc.sync.dma_start(out=outd[:], in_=out_sb[:])

    _split_sync_waits(nc)
    return nc


def _prep_host(x, rij, vij, src, dst):
    """Sort edges by dst, shard by dst range, pad per 128-node block."""
    src = np.asarray(src).astype(np.int64)
    dst = np.asarray(dst).astype(np.int64)
    rij = np.asarray(rij, dtype=np.float32)
    vij = np.asarray(vij, dtype=np.float32)

    order = np.argsort(dst, kind="stable")
    ds = dst[order]
    ss = src[order]
    rs = rij[order]
    vs = vij[order]

    gblk = ds // 128                      # global block id, 0..391
    nblk_tot = (NFULL // 128)             # 392
    cnt = np.bincount(gblk, minlength=nblk_tot)
    t_blk = int(np.ceil(cnt.max() / 128))
    start = np.concatenate([[0], np.cumsum(cnt)[:-1]])
    within = np.arange(len(ds)) - start[gblk]

    epd = NB * t_blk * 128
    dev = gblk // NB
    slot = (gblk % NB) * (t_blk * 128) + within

    srcA = np.zeros((NCORES, epd), np.int32)
    dstmA = np.zeros((NCORES, epd), np.float32)
    rijA = np.full((NCORES, epd), 2.0 * CUTOFF, np.float32)
    vijA = np.zeros((NCORES, epd, 3), np.float32)
    srcA[dev, slot] = ss
    dstmA[dev, slot] = (ds % 128).astype(np.float32)
    rijA[dev, slot] = rs
    vijA[dev, slot] = vs

    nt = NB * t_blk
    ins = []
    xf = np.zeros((NFULL, F), np.float32)
    xf[:N] = np.asarray(x, dtype=np.float32)
    for d in range(NCORES):
        ins.append({
            "xT": np.ascontiguousarray(
                xf[d * NPD:(d + 1) * NPD].T),
            "srcT": np.ascontiguousarray(
                srcA[d].reshape(nt, 128).T),
            "dstmT": np.ascontiguousarray(
                dstmA[d].reshape(nt, 128).T),
            "rijT": np.ascontiguousarray(
                rijA[d].reshape(nt, 128).T),
            "vijT": np.ascontiguousarray(
                vijA[d].reshape(nt, 128, 3).transpose(2, 0, 1)
                .reshape(3 * nt, 128).T),
        })
    return ins, t_blk


def kernel(x, rij, vij, src, dst, W1, b1, W2, b2, W1b, b1b, W2b, b2b,
           w_mix, b_mix):
    import time

    ins, t_blk = _prep_host(x, rij, vij, src, dst)

    shared = {
        "w1": np.asarray(W1, np.float32),
        "w1b": np.asarray(W1b, np.float32),
        "w2": np.asarray(W2, np.float32),
        "w2b": np.asarray(W2b, np.float32),
        "b1c": np.asarray(b1, np.float32).reshape(F, 1),
        "b1bc": np.asarray(b1b, np.float32).reshape(F, 1),
        "b2bc": np.tile(
            np.concatenate([np.asarray(b2, np.float32),
                            np.asarray(b2b, np.float32)])[None, :],
            (128, 1)),
        "wbmix": np.tile(
            np.concatenate([np.asarray(w_mix, np.float32),
                            np.asarray(b_mix, np.float32),
                            np.array([np.pi / 2.0, np.pi / CUTOFF],
                                     np.float32)])[None, :],
            (128, 1)),
        "iota": np.tile(np.arange(128, dtype=np.float32)[None, :], (128, 1)),
    }
    for m in ins:
        m.update(shared)

    t0 = time.time()
    nc = _build_nc(t_blk)
    LAST_RUN_INFO["build_s"] = time.time() - t0

    t0 = time.time()
    res = run_bass_kernel_spmd(nc, ins, core_ids=list(range(NCORES)))
    LAST_RUN_INFO["run_s"] = time.time() - t0
    LAST_RUN_INFO["t_blk"] = t_blk
    LAST_RUN_INFO["nc"] = nc
    LAST_RUN_INFO["in_maps"] = ins

    # outd[p, g*96 + k*32 + d'] for node g*128+p
    full = np.empty((NFULL, D, 3), np.float32)
    for d in range(NCORES):
        od = res.results[d]["outd"].reshape(128, NB, 3, D)
        full[d * NPD:(d + 1) * NPD] = (
            od.transpose(1, 0, 3, 2).reshape(NPD, D, 3))
    return full[:N]


# revision 3
# speedup vs baseline: 2.0832x; 2.0832x over previous
"""Trainium2 Bass kernel for the Dipole GNN message-passing layer.

Strategy (8 NeuronCores):
  * Node MLPs (q, q2) are computed sharded over nodes (6272 nodes/core,
    host supplies x pre-transposed per shard), then the per-node feature
    table qq = [q | q2] (fp32, [50176, 64]) is AllGathered so every core
    holds the full table in DRAM.
  * Edges are sharded by dst range: host sorts edges by dst, so each core
    owns all edges landing in its 6272-node range and produces a disjoint
    slice of the output (no all-reduce needed).
  * Within a core, edges are grouped into 49 blocks of 128 dst nodes and
    padded per block to a fixed tile count. Each 128-edge tile:
      - gathers qq[src] rows via indirect DMA (128 rows/call),
      - builds M12[e, k*64+b*32+d] = vij[e,k]*c(rij[e])*qq[src[e], b*32+d]
        with three per-partition tensor_scalar ops,
      - builds a one-hot dst-selection matrix via is_equal against an
        iota row, and accumulates onehot.T @ M12 into the block's PSUM
        tile ([128 nodes, 192]).
  * Cross product + mix Dense(3,1) run as bulk DVE ops; host reassembles
    the [N, 32, 3] output from the per-core slices.
"""

import sys

sys.path.insert(0, "/opt/trn_rl_repo")

import numpy as np

import concourse.bass as bass
import concourse.mybir as mybir
import concourse.tile as tile
from concourse.bass_utils import run_bass_kernel_spmd

N, E, F, D = 50000, 800000, 128, 32
CUTOFF = 5.0
NCORES = 8
NPD = 6272          # nodes per device (49 blocks of 128)
NB = NPD // 128     # 49 blocks
NFULL = NPD * NCORES  # 50176
DQ = 2 * D          # 64 = [q | q2]

LAST_RUN_INFO = {}


def _split_sync_waits(nc, limit=1):
    """The walrus build here encodes very few sync waits per instruction
    (Drain: 1). Hoist excess on_wait entries onto single-wait NoOps
    inserted just before the instruction on the same engine."""
    ctr = 0
    for fn in nc.m.functions:
        for blk in fn.blocks:
            out = []
            for ins in blk.instructions:
                si = ins.sync_info
                if si is not None and len(si.on_wait) > limit:
                    waits = list(si.on_wait)
                    for w in waits[limit:]:
                        ctr += 1
                        nop = mybir.InstNoOp(
                            name=f"I-swfix-{ctr}", ins=[], outs=[]
                        )
                        nop.engine = ins.engine
                        nop.sync_info = mybir.SyncInfo(
                            on_wait=[w], on_update=[]
                        )
                        out.append(nop)
                    si.on_wait = waits[:limit]
                out.append(ins)
            blk.instructions = out
    return nc


def _build_nc(t_blk):
    NT = NB * t_blk  # tiles per device
    f32 = mybir.dt.float32
    nc = bass.Bass()

    xT = nc.dram_tensor("xT", [F, NPD], f32, kind="ExternalInput")
    w1 = nc.dram_tensor("w1", [F, F], f32, kind="ExternalInput")
    w1b = nc.dram_tensor("w1b", [F, F], f32, kind="ExternalInput")
    w2 = nc.dram_tensor("w2", [F, D], f32, kind="ExternalInput")
    w2b = nc.dram_tensor("w2b", [F, D], f32, kind="ExternalInput")
    b1c = nc.dram_tensor("b1c", [F, 1], f32, kind="ExternalInput")
    b1bc = nc.dram_tensor("b1bc", [F, 1], f32, kind="ExternalInput")
    b2bc = nc.dram_tensor("b2bc", [128, DQ], f32, kind="ExternalInput")
    wbmix = nc.dram_tensor("wbmix", [128, 6], f32, kind="ExternalInput")
    iota = nc.dram_tensor("iota", [128, 128], f32, kind="ExternalInput")
    srcT = nc.dram_tensor("srcT", [128, NT], mybir.dt.int32, kind="ExternalInput")
    dstmT = nc.dram_tensor("dstmT", [128, NT], f32, kind="ExternalInput")
    rijT = nc.dram_tensor("rijT", [128, NT], f32, kind="ExternalInput")
    vijT = nc.dram_tensor("vijT", [128, 3 * NT], f32, kind="ExternalInput")
    outd = nc.dram_tensor("outd", [128, NB * 96], f32, kind="ExternalOutput")

    with tile.TileContext(nc) as tc:
        with tc.tile_pool(name="dram", bufs=1, space="DRAM") as dram, \
             tc.tile_pool(name="persist", bufs=1) as pp:
            qq_own = dram.tile([NPD, DQ], f32)
            qq_full = dram.tile([NFULL, DQ], f32, addr_space="Shared")

            # ---- persistent SBUF loads (edge arrays + constants) ----
            src_sb = pp.tile([128, NT], mybir.dt.int32)
            dstm_sb = pp.tile([128, NT], f32)
            rij_sb = pp.tile([128, NT], f32)
            vij_sb = pp.tile([128, 3 * NT], f32)
            iota_sb = pp.tile([128, 128], f32)
            b2_sb = pp.tile([128, DQ], f32)
            wb_sb = pp.tile([128, 6], f32)
            nc.sync.dma_start(out=src_sb[:], in_=srcT[:])
            nc.sync.dma_start(out=dstm_sb[:], in_=dstmT[:])
            nc.sync.dma_start(out=rij_sb[:], in_=rijT[:])
            nc.sync.dma_start(out=vij_sb[:], in_=vijT[:])
            nc.sync.dma_start(out=iota_sb[:], in_=iota[:])
            nc.sync.dma_start(out=b2_sb[:], in_=b2bc[:])
            nc.sync.dma_start(out=wb_sb[:], in_=wbmix[:])

            # vijc[p, k*NT+g] = vij * c(rij), shared by both branches
            vijc_sb = pp.tile([128, 3 * NT], f32)
            c_sb = pp.tile([128, NT], f32)
            mask_sb = pp.tile([128, NT], f32)
            mu_sb = pp.tile([128, NB * 192], f32)
            out_sb = pp.tile([128, NB * 96], f32)

            # ---- stage 1: q/q2 for own node slice ----
            with tc.tile_pool(name="s1", bufs=1) as s1, \
                 tc.tile_pool(name="s1p", bufs=2, space="PSUM") as s1p, \
                 tc.tile_pool(name="s1w", bufs=3) as s1w:
                xT_sb = s1.tile([F, NPD], f32)
                h1_sb = s1.tile([F, NPD], f32)
                h2_sb = s1.tile([F, NPD], f32)
                w1_sb = s1.tile([F, F], f32, tag="w1")
                w1b_sb = s1.tile([F, F], f32, tag="w1b")
                w2_sb = s1.tile([F, D], f32, tag="w2")
                w2b_sb = s1.tile([F, D], f32, tag="w2b")
                b1_sb = s1.tile([F, 1], f32, tag="b1")
                b1b_sb = s1.tile([F, 1], f32, tag="b1b")
                nc.sync.dma_start(out=xT_sb[:], in_=xT[:])
                nc.sync.dma_start(out=w1_sb[:], in_=w1[:])
                nc.sync.dma_start(out=w1b_sb[:], in_=w1b[:])
                nc.sync.dma_start(out=w2_sb[:], in_=w2[:])
                nc.sync.dma_start(out=w2b_sb[:], in_=w2b[:])
                nc.sync.dma_start(out=b1_sb[:], in_=b1c[:])
                nc.sync.dma_start(out=b1b_sb[:], in_=b1bc[:])

                silu = mybir.ActivationFunctionType.Silu
                # layer 1, both branches: hT = silu(W.T @ xT + b)
                for wsb, bsb, hsb in ((w1_sb, b1_sb, h1_sb),
                                      (w1b_sb, b1b_sb, h2_sb)):
                    c0 = 0
                    while c0 < NPD:
                        cw = min(512, NPD - c0)
                        ph = s1p.tile([128, 512], f32, tag="ph")
                        nc.tensor.matmul(
                            out=ph[:, :cw], lhsT=wsb[:],
                            rhs=xT_sb[:, c0:c0 + cw],
                            start=True, stop=True,
                        )
                        nc.scalar.activation(
                            out=hsb[:, c0:c0 + cw], in_=ph[:, :cw],
                            func=silu, bias=bsb[:],
                        )
                        c0 += cw
                # layer 2 per 128-node chunk -> qq rows
                for cnode in range(NB):
                    sl = slice(cnode * 128, (cnode + 1) * 128)
                    pq = s1p.tile([128, DQ], f32, tag="pq")
                    nc.tensor.matmul(out=pq[:, 0:D], lhsT=h1_sb[:, sl],
                                     rhs=w2_sb[:], start=True, stop=True)
                    nc.tensor.matmul(out=pq[:, D:DQ], lhsT=h2_sb[:, sl],
                                     rhs=w2b_sb[:], start=True, stop=True)
                    zq = s1w.tile([128, DQ], f32, tag="zq")
                    qt = s1w.tile([128, DQ], f32, tag="qt")
                    nc.vector.tensor_tensor(out=zq[:], in0=pq[:],
                                            in1=b2_sb[:],
                                            op=mybir.AluOpType.add)
                    nc.scalar.activation(out=qt[:], in_=zq[:], func=silu)
                    nc.sync.dma_start(out=qq_own[sl, :], in_=qt[:])

            # ---- distribute qq to all cores ----
            nc.gpsimd.collective_compute(
                "AllGather", mybir.AluOpType.bypass,
                replica_groups=[list(range(NCORES))],
                ins=[qq_own.opt()], outs=[qq_full.opt()],
            )

            # ---- edge-stage prep: cutoff + vij*c ----
            half = NT // 2
            for s0, s1e in ((0, half), (half, NT)):
                nc.scalar.activation(
                    out=c_sb[:, s0:s1e], in_=rij_sb[:, s0:s1e],
                    func=mybir.ActivationFunctionType.Sin,
                    scale=wb_sb[:, 5:6], bias=wb_sb[:, 4:5],
                )
            nc.vector.tensor_scalar(
                out=mask_sb[:], in0=rij_sb[:], scalar1=CUTOFF, scalar2=None,
                op0=mybir.AluOpType.is_lt,
            )
            # c = sin(pi/2 - pi*r/10)^2 * [r < cutoff]  (arg stays within
            # the ACT Sin table's accurate range [-pi/2, pi/2])
            nc.vector.tensor_tensor(out=c_sb[:], in0=c_sb[:], in1=c_sb[:],
                                    op=mybir.AluOpType.mult)
            nc.vector.tensor_tensor(out=c_sb[:], in0=c_sb[:], in1=mask_sb[:],
                                    op=mybir.AluOpType.mult)
            for k in range(3):
                nc.vector.tensor_tensor(
                    out=vijc_sb[:, k * NT:(k + 1) * NT],
                    in0=vij_sb[:, k * NT:(k + 1) * NT],
                    in1=c_sb[:], op=mybir.AluOpType.mult,
                )

            # ---- edge loop: gather + one-hot matmul segment sum ----
            with tc.tile_pool(name="ep", bufs=6) as ep, \
                 tc.tile_pool(name="epp", bufs=2, space="PSUM") as epp:
                for b in range(NB):
                    pmu = epp.tile([128, 192], f32, tag="pmu")
                    for t in range(t_blk):
                        g = b * t_blk + t
                        qg = ep.tile([128, DQ], f32, tag="qg")
                        nc.gpsimd.indirect_dma_start(
                            out=qg[:], out_offset=None, in_=qq_full[:],
                            in_offset=bass.IndirectOffsetOnAxis(
                                ap=src_sb[:, g:g + 1], axis=0),
                        )
                        oh = ep.tile([128, 128], f32, tag="oh")
                        nc.vector.tensor_scalar(
                            out=oh[:], in0=iota_sb[:],
                            scalar1=dstm_sb[:, g:g + 1], scalar2=None,
                            op0=mybir.AluOpType.is_equal,
                        )
                        m12 = ep.tile([128, 192], f32, tag="m12")
                        for k in range(3):
                            nc.vector.tensor_scalar(
                                out=m12[:, k * DQ:(k + 1) * DQ], in0=qg[:],
                                scalar1=vijc_sb[:, k * NT + g:k * NT + g + 1],
                                scalar2=None, op0=mybir.AluOpType.mult,
                            )
                        nc.tensor.matmul(
                            out=pmu[:], lhsT=oh[:], rhs=m12[:],
                            start=(t == 0), stop=(t == t_blk - 1),
                        )
                    nc.scalar.copy(out=mu_sb[:, b * 192:(b + 1) * 192],
                                   in_=pmu[:])

            # ---- finalize: cross product + mix ----
            with tc.tile_pool(name="fin", bufs=1) as fin:
                mu3v = mu_sb[:].rearrange("p (g c) -> p g c", c=192)
                out3v = out_sb[:].rearrange("p (g c) -> p g c", c=96)

                def muv(k):
                    return mu3v[:, :, k * DQ:k * DQ + D]

                def mu2v(k):
                    return mu3v[:, :, k * DQ + D:k * DQ + DQ]

                W = NB * D
                for k in range(3):
                    k1, k2 = (k + 1) % 3, (k + 2) % 3
                    t1 = fin.tile([128, W], f32, tag="t1")
                    t2 = fin.tile([128, W], f32, tag="t2")
                    o3 = fin.tile([128, W], f32, tag="o3")
                    o1 = fin.tile([128, W], f32, tag="o1")
                    o2 = fin.tile([128, W], f32, tag="o2")
                    mul = mybir.AluOpType.mult
                    nc.vector.tensor_tensor(out=t1[:], in0=muv(k1), in1=mu2v(k2), op=mul)
                    nc.vector.tensor_tensor(out=t2[:], in0=muv(k2), in1=mu2v(k1), op=mul)
                    nc.vector.tensor_tensor(out=t1[:], in0=t1[:], in1=t2[:],
                                            op=mybir.AluOpType.subtract)
                    # o3 = w2*mu3 + b ; o1 = w0*mu ; o2 = w1*mu2
                    nc.vector.tensor_scalar(out=o3[:], in0=t1[:],
                                            scalar1=wb_sb[:, 2:3],
                                            scalar2=wb_sb[:, 3:4],
                                            op0=mul, op1=mybir.AluOpType.add)
                    nc.vector.tensor_scalar(out=o1[:], in0=muv(k),
                                            scalar1=wb_sb[:, 0:1],
                                            scalar2=None, op0=mul)
                    nc.vector.tensor_scalar(out=o2[:], in0=mu2v(k),
                                            scalar1=wb_sb[:, 1:2],
                                            scalar2=None, op0=mul)
                    nc.vector.tensor_tensor(out=o1[:], in0=o1[:], in1=o2[:],
                                            op=mybir.AluOpType.add)
                    nc.vector.tensor_tensor(out=out3v[:, :, k * D:(k + 1) * D],
                                            in0=o1[:], in1=o3[:],
                                            op=mybir.AluOpType.add)
                nc.sync.dma_start(out=outd[:], in_=out_sb[:])

    _split_sync_waits(nc)
    return nc


def _prep_host(x, rij, vij, src, dst):
    """Sort edges by dst, shard by dst range, pad per 128-node block."""
    src = np.asarray(src).astype(np.int64)
    dst = np.asarray(dst).astype(np.int64)
    rij = np.asarray(rij, dtype=np.float32)
    vij = np.asarray(vij, dtype=np.float32)

    order = np.argsort(dst, kind="stable")
    ds = dst[order]
    ss = src[order]
    rs = rij[order]
    vs = vij[order]

    gblk = ds // 128                      # global block id, 0..391
    nblk_tot = (NFULL // 128)             # 392
    cnt = np.bincount(gblk, minlength=nblk_tot)
    t_blk = int(np.ceil(cnt.max() / 128))
    start = np.concatenate([[0], np.cumsum(cnt)[:-1]])
    within = np.arange(len(ds)) - start[gblk]

    epd = NB * t_blk * 128
    dev = gblk // NB
    slot = (gblk % NB) * (t_blk * 128) + within

    srcA = np.zeros((NCORES, epd), np.int32)
    dstmA = np.zeros((NCORES, epd), np.float32)
    rijA = np.full((NCORES, epd), 2.0 * CUTOFF, np.float32)
    vijA = np.zeros((NCORES, epd, 3), np.float32)
    srcA[dev, slot] = ss
    dstmA[dev, slot] = (ds % 128).astype(np.float32)
    rijA[dev, slot] = rs
    vijA[dev, slot] = vs

    nt = NB * t_blk
    ins = []
    xf = np.zeros((NFULL, F), np.float32)
    xf[:N] = np.asarray(x, dtype=np.float32)
    for d in range(NCORES):
        ins.append({
            "xT": np.ascontiguousarray(
                xf[d * NPD:(d + 1) * NPD].T),
            "srcT": np.ascontiguousarray(
                srcA[d].reshape(nt, 128).T),
            "dstmT": np.ascontiguousarray(
                dstmA[d].reshape(nt, 128).T),
            "rijT": np.ascontiguousarray(
                rijA[d].reshape(nt, 128).T),
            "vijT": np.ascontiguousarray(
                vijA[d].reshape(nt, 128, 3).transpose(2, 0, 1)
                .reshape(3 * nt, 128).T),
        })
    return ins, t_blk


def kernel(x, rij, vij, src, dst, W1, b1, W2, b2, W1b, b1b, W2b, b2b,
           w_mix, b_mix):
    import time

    ins, t_blk = _prep_host(x, rij, vij, src, dst)

    shared = {
        "w1": np.asarray(W1, np.float32),
        "w1b": np.asarray(W1b, np.float32),
        "w2": np.asarray(W2, np.float32),
        "w2b": np.asarray(W2b, np.float32),
        "b1c": np.asarray(b1, np.float32).reshape(F, 1),
        "b1bc": np.asarray(b1b, np.float32).reshape(F, 1),
        "b2bc": np.tile(
            np.concatenate([np.asarray(b2, np.float32),
                            np.asarray(b2b, np.float32)])[None, :],
            (128, 1)),
        "wbmix": np.tile(
            np.concatenate([np.asarray(w_mix, np.float32),
                            np.asarray(b_mix, np.float32),
                            np.array([np.pi / 2.0, -np.pi / (2.0 * CUTOFF)],
                                     np.float32)])[None, :],
            (128, 1)),
        "iota": np.tile(np.arange(128, dtype=np.float32)[None, :], (128, 1)),
    }
    for m in ins:
        m.update(shared)

    t0 = time.time()
    nc = _build_nc(t_blk)
    LAST_RUN_INFO["build_s"] = time.time() - t0

    t0 = time.time()
    res = run_bass_kernel_spmd(nc, ins, core_ids=list(range(NCORES)))
    LAST_RUN_INFO["run_s"] = time.time() - t0
    LAST_RUN_INFO["t_blk"] = t_blk
    LAST_RUN_INFO["nc"] = nc
    LAST_RUN_INFO["in_maps"] = ins

    # outd[p, g*96 + k*32 + d'] for node g*128+p
    full = np.empty((NFULL, D, 3), np.float32)
    for d in range(NCORES):
        od = res.results[d]["outd"].reshape(128, NB, 3, D)
        full[d * NPD:(d + 1) * NPD] = (
            od.transpose(1, 0, 3, 2).reshape(NPD, D, 3))
    return full[:N]
